# revision 8
# baseline (speedup 1.0000x reference)
"""CTPN target assignment kernel for 8 Trainium2 NeuronCores.

Data-parallel: one image per core. The device computes the dominant
O(G*M) work per image: the full [50, 61440] IoU matrix (tiled as
[128 partitions x 480] over anchors, GT dim unrolled), per-GT row
maxes (gmax), per-anchor column maxes + argmax (amax/aidx), and the
per-anchor GT-argmax key (keyA) that marks "best anchor for GT g"
columns. Those four small per-anchor/per-GT arrays are returned and
the O(pos_cnt) sampling/assembly (top-64 by precomputed threefry
priorities, negative sampling, box regression on <=64 rows) finishes
on host, mirroring the reference bit-exactly on the selection path.
"""

import sys
import numpy as np

if "/opt/trn_rl_repo" not in sys.path:
    sys.path.insert(0, "/opt/trn_rl_repo")

import concourse.bass as bass
import concourse.bacc as bacc
import concourse.mybir as mybir
from concourse.tile import TileContext
from concourse import bass_utils
from concourse.masks import make_identity

B, G, M = 8, 50, 61440
P, F = 128, 480  # anchor tiling: m = p*F + f
K_POS = K_NEG = 64
NEG_T = 256
F07 = float(np.float32(0.7))
ALU = mybir.AluOpType
ACTF = mybir.ActivationFunctionType
DT = mybir.dt


# ---------------- threefry (bit-exact numpy port of jax partitionable PRNG) ----
def _threefry2x32(k0, k1, x0, x1):
    rot = [(13, 15, 26, 6), (17, 29, 16, 24)]
    ks0, ks1 = np.uint32(k0), np.uint32(k1)
    ks = [ks0, ks1, np.uint32(ks0 ^ ks1 ^ np.uint32(0x1BD11BDA))]
    x0 = (x0 + ks0).astype(np.uint32)
    x1 = (x1 + ks1).astype(np.uint32)
    for i in range(5):
        for r in rot[i % 2]:
            x0 = (x0 + x1).astype(np.uint32)
            x1 = ((x1 << np.uint32(r)) | (x1 >> np.uint32(32 - r))).astype(np.uint32)
            x1 = (x1 ^ x0).astype(np.uint32)
        x0 = (x0 + ks[(i + 1) % 3]).astype(np.uint32)
        x1 = (x1 + ks[(i + 2) % 3] + np.uint32(i + 1)).astype(np.uint32)
    return x0, x1


def _tf_bits(k0, k1, n):
    idx = np.arange(n, dtype=np.uint64)
    hi = (idx >> np.uint64(32)).astype(np.uint32)
    lo = (idx & np.uint64(0xFFFFFFFF)).astype(np.uint32)
    b1, b2 = _threefry2x32(k0, k1, hi, lo)
    return (b1 ^ b2).astype(np.uint32)


def _tf_split(k0, k1, num):
    idx = np.arange(num, dtype=np.uint64)
    hi = (idx >> np.uint64(32)).astype(np.uint32)
    lo = (idx & np.uint64(0xFFFFFFFF)).astype(np.uint32)
    b1, b2 = _threefry2x32(k0, k1, hi, lo)
    return np.stack([b1, b2], axis=1)


def _tf_uniform(k0, k1, n):
    bits = _tf_bits(k0, k1, n)
    f = ((bits >> np.uint32(9)) | np.uint32(0x3F800000)).view(np.float32)
    return np.maximum(np.float32(0.0), (f - np.float32(1.0)).astype(np.float32))


_PRNG = None


def _prng_consts():
    """UP[b] = uniform(kp,(G*M,)), neg order = argsort(-UN[b])[:NEG_T]."""
    global _PRNG
    if _PRNG is None:
        keys = _tf_split(np.uint32(0), np.uint32(42), B)
        UP, NEG = [], []
        for b in range(B):
            kp_kn = _tf_split(keys[b][0], keys[b][1], 2)
            UP.append(_tf_uniform(kp_kn[0][0], kp_kn[0][1], G * M))
            un = _tf_uniform(kp_kn[1][0], kp_kn[1][1], M)
            NEG.append(np.argsort(-un, kind="stable")[:NEG_T].astype(np.int64))
        _PRNG = (UP, NEG)
    return _PRNG


# ---------------- device program -------------------------------------------
def _build_program():
    nc = bacc.Bacc("TRN2", target_bir_lowering=False, debug=False)
    gtb = nc.dram_tensor("gtb", [G, 5], DT.float32, kind="ExternalInput")
    anc = nc.dram_tensor("anc", [M, 4], DT.float32, kind="ExternalInput")
    amax_d = nc.dram_tensor("amax_d", [M], DT.float32, kind="ExternalOutput")
    aidx_d = nc.dram_tensor("aidx_d", [M], DT.float32, kind="ExternalOutput")
    keyA_d = nc.dram_tensor("keyA_d", [M], DT.float32, kind="ExternalOutput")
    gmax_d = nc.dram_tensor("gmax_d", [G], DT.float32, kind="ExternalOutput")

    with TileContext(nc) as tc:
        with (
            tc.tile_pool(name="per", bufs=1) as per,
            tc.tile_pool(name="wrk", bufs=3) as wrk,
            tc.tile_pool(name="ps", bufs=2, space="PSUM") as ps,
        ):
            f32 = DT.float32
            packed = per.tile([P, F * 4], f32)
            nc.sync.dma_start(
                packed[:], anc[:].rearrange("(p f) c -> p (f c)", p=P)
            )
            pk3 = packed[:].rearrange("p (f c) -> p f c", c=4)
            AY1 = per.tile([P, F], f32, tag="ay1")
            AX1 = per.tile([P, F], f32, tag="ax1")
            AY2 = per.tile([P, F], f32, tag="ay2")
            AX2 = per.tile([P, F], f32, tag="ax2")
            for t, c in ((AY1, 0), (AX1, 1), (AY2, 2), (AX2, 3)):
                nc.vector.tensor_copy(t[:], pk3[:, :, c])
            Aa = per.tile([P, F], f32, tag="aa")
            tw = wrk.tile([P, F], f32, tag="tw")
            nc.vector.tensor_sub(tw[:], AX2[:], AX1[:])
            nc.vector.tensor_sub(Aa[:], AY2[:], AY1[:])
            nc.vector.tensor_mul(Aa[:], tw[:], Aa[:])

            # gt prep -> [50,8] -> transpose -> broadcast tiles [128,50]
            g50 = per.tile([G, 5], f32, tag="g50")
            nc.sync.dma_start(g50[:], gtb[:])
            ext = per.tile([G, 8], f32, tag="ext")
            nc.vector.memset(ext[:], 0.0)
            for c in range(4):
                nc.vector.tensor_copy(ext[:, c : c + 1], g50[:, c : c + 1])
            w1 = wrk.tile([G, 1], f32, tag="w1")
            nc.vector.tensor_sub(w1[:], g50[:, 3:4], g50[:, 1:2])
            nc.vector.tensor_sub(ext[:, 4:5], g50[:, 2:3], g50[:, 0:1])
            nc.vector.tensor_mul(ext[:, 4:5], w1[:], ext[:, 4:5])
            nc.vector.tensor_scalar(
                ext[:, 5:6], g50[:, 4:5], 0.0, None, ALU.not_equal
            )
            ident = per.tile([P, P], f32, tag="ident")
            make_identity(nc, ident[:])
            extT_ps = ps.tile([8, G], f32, space="PSUM")
            nc.tensor.transpose(extT_ps[:], ext[:], ident[:G, :G])
            t8 = per.tile([8, G], f32, tag="t8")
            nc.vector.tensor_copy(t8[:], extT_ps[:])
            names = ["gy1b", "gx1b", "gy2b", "gx2b", "agb", "validb"]
            bcast = {}
            for k, nm in enumerate(names):
                r0 = per.tile([1, G], f32, tag=nm + "_r0")
                nc.sync.dma_start(r0[:], t8[k : k + 1, :])
                bt = per.tile([P, G], f32, tag=nm)
                nc.gpsimd.partition_broadcast(bt[:], r0[:])
                bcast[nm] = bt
            gy1b, gx1b = bcast["gy1b"], bcast["gx1b"]
            gy2b, gx2b = bcast["gy2b"], bcast["gx2b"]
            agb, validb = bcast["agb"], bcast["validb"]

            iou_big = per.tile([P, F * G], f32, tag="ioubig")
            amax = per.tile([P, F], f32, tag="amax")
            aidx = per.tile([P, F], f32, tag="aidx")
            keyA = per.tile([P, F], f32, tag="keya")
            gcols = per.tile([P, G], f32, tag="gcols")
            nc.vector.memset(amax[:], 0.0)
            nc.vector.memset(aidx[:], 0.0)
            nc.vector.memset(keyA[:], 0.0)

            import os as _os
            _GRUN = int(_os.environ.get('KPROBE_G', G))
            _PH = int(_os.environ.get('KPROBE_PH', 2))
            for g in range(_GRUN):
                sl = slice(g * F, (g + 1) * F)
                iou = iou_big[:, sl]
                mnx = wrk.tile([P, F], f32, tag="mnx")
                nxw = wrk.tile([P, F], f32, tag="nxw")
                mny = wrk.tile([P, F], f32, tag="mny")
                nyh = wrk.tile([P, F], f32, tag="nyh")
                iwp = wrk.tile([P, F], f32, tag="iwp")
                ihp = wrk.tile([P, F], f32, tag="ihp")
                uni = wrk.tile([P, F], f32, tag="uni")
                rcp = wrk.tile([P, F], f32, tag="rcp")
                cmp = wrk.tile([P, F], f32, tag="cmp")
                nc.vector.tensor_scalar(mnx[:], AX2[:], gx2b[:, g : g + 1], None, ALU.min)
                nc.vector.scalar_tensor_tensor(
                    nxw[:], AX1[:], gx1b[:, g : g + 1], mnx[:], ALU.max, ALU.subtract
                )
                nc.vector.tensor_scalar(mny[:], AY2[:], gy2b[:, g : g + 1], None, ALU.min)
                nc.vector.scalar_tensor_tensor(
                    nyh[:], AY1[:], gy1b[:, g : g + 1], mny[:], ALU.max, ALU.subtract
                )
                # iw = relu(-nxw), ih = relu(-nyh) on ACT (parallel engine)
                nc.scalar.activation(iwp[:], nxw[:], ACTF.Relu, 0.0, -1.0)
                nc.scalar.activation(ihp[:], nyh[:], ACTF.Relu, 0.0, -1.0)
                inter = mnx  # reuse
                nc.vector.tensor_mul(inter[:], iwp[:], ihp[:])
                nc.vector.scalar_tensor_tensor(
                    uni[:], Aa[:], agb[:, g : g + 1], inter[:], ALU.add, ALU.subtract
                )
                nc.vector.reciprocal_approx_fast(rcp[:], uni[:])
                nc.vector.scalar_tensor_tensor(
                    iou[:], inter[:], validb[:, g : g + 1], rcp[:], ALU.mult, ALU.mult
                )
                nc.vector.tensor_reduce(
                    gcols[:, g : g + 1], iou[:], mybir.AxisListType.X, ALU.max
                )
                nc.vector.tensor_tensor(cmp[:], iou[:], amax[:], ALU.is_gt)
                nc.vector.tensor_max(amax[:], amax[:], iou[:])
                nc.vector.scalar_tensor_tensor(
                    aidx[:], cmp[:], float(g), aidx[:], ALU.mult, ALU.max
                )

            if _PH < 2:
                nc.sync.dma_start(amax_d[:].rearrange("(p f) -> p f", p=P), amax[:])
                nc.sync.dma_start(aidx_d[:].rearrange("(p f) -> p f", p=P), aidx[:])
                nc.sync.dma_start(keyA_d[:].rearrange("(p f) -> p f", p=P), keyA[:])
                nc.sync.dma_start(gmax_d[:].rearrange("(p f) -> p f", f=1), gcols[:G, 0:1])
                nc.compile()
                return nc
            # gmax: [128,50] -> T -> [50,128] -> reduce X
            gT_ps = ps.tile([G, P], f32, space="PSUM")
            nc.tensor.transpose(gT_ps[:], gcols[:], ident[:])
            gmax = per.tile([G, 1], f32, tag="gmax")
            nc.vector.tensor_reduce(gmax[:], gT_ps[:], mybir.AxisListType.X, ALU.max)
            # broadcast gmax -> [128,50]; select -1 for invalid rows
            gmT_ps = ps.tile([1, G], f32, space="PSUM")
            nc.tensor.transpose(gmT_ps[:], gmax[:], ident[:G, :G])
            grow = per.tile([1, G], f32, tag="grow")
            nc.vector.tensor_copy(grow[:], gmT_ps[:])
            gmaxb = per.tile([P, G], f32, tag="gmaxb")
            nc.gpsimd.partition_broadcast(gmaxb[:], grow[:])
            gsel = per.tile([P, G], f32, tag="gsel")
            nc.vector.tensor_mul(gsel[:], gmaxb[:], validb[:])
            nc.vector.scalar_tensor_tensor(
                gsel[:], validb[:], -1.0, gsel[:], ALU.add, ALU.add
            )
            for g in range(_GRUN):
                e = wrk.tile([P, F], f32, tag="e2")
                nc.vector.tensor_scalar(
                    e[:], iou_big[:, g * F : (g + 1) * F], gsel[:, g : g + 1], None,
                    ALU.is_equal,
                )
                nc.vector.scalar_tensor_tensor(
                    keyA[:], e[:], float(g + 1), keyA[:], ALU.mult, ALU.max
                )

            nc.sync.dma_start(amax_d[:].rearrange("(p f) -> p f", p=P), amax[:])
            nc.sync.dma_start(aidx_d[:].rearrange("(p f) -> p f", p=P), aidx[:])
            nc.sync.dma_start(keyA_d[:].rearrange("(p f) -> p f", p=P), keyA[:])
            nc.sync.dma_start(gmax_d[:].rearrange("(p f) -> p f", f=1), gmax[:])
    nc.compile()
    return nc


_NC = None


def _get_nc():
    global _NC
    if _NC is None:
        _NC = _build_program()
    return _NC


# ---------------- host assembly (mirrors verified golden model) -------------
def _finish_one(gt, gcls_b, anc, vidx, up, negids, amax, aidx, keyA, gmax):
    f32 = np.float32
    gt = np.asarray(gt, f32)
    hot = amax >= f32(F07)
    entA = keyA > 0
    gA = np.where(entA, keyA - 1, 0).astype(np.int64)
    vA = gmax[gA]
    dup = entA & hot & (vA == amax)
    mB = np.where(hot)[0]
    mA = np.where(entA & ~dup)[0]
    ents_g = np.concatenate([aidx[mB].astype(np.int64), gA[mA]])
    ents_m = np.concatenate([mB, mA])
    ents_v = np.concatenate([amax[mB], vA[mA]])
    pos_cnt = len(ents_g)
    min_iou = ents_v.min()
    mean_iou = f32(ents_v.sum(dtype=f32) / f32(pos_cnt))
    prio = up[ents_g * M + ents_m]
    order = np.argsort(-prio, kind="stable")[:K_POS]
    sg, sm = ents_g[order], ents_m[order]
    nsel = len(sg)  # == 64 (pos_cnt >= 64 for this data)

    flags = (amax[negids] < f32(0.5)) & ~(entA[negids] | hot[negids])
    njs = negids[flags][:K_NEG]

    deltas = np.zeros((128, 4), f32)
    cls_out = np.zeros((128, 2), f32)
    indices = np.zeros((128, 2), np.int32)
    a = anc[sm]
    gg = gt[sg]
    hh = (a[:, 2] - a[:, 0]).astype(f32)
    gth = (gg[:, 2] - gg[:, 0]).astype(f32)
    dy = ((((gg[:, 2] + gg[:, 0]) * f32(0.5)) - ((a[:, 2] + a[:, 0]) * f32(0.5))) / hh).astype(f32)
    dh = np.log((gth / hh).astype(f32)).astype(f32)
    ww = (a[:, 3] - a[:, 1]).astype(f32)
    dx = (((((gg[:, 3] + gg[:, 1]) * f32(0.5)) - ((a[:, 3] + a[:, 1]) * f32(0.5))) * f32(2.0)) / ww).astype(f32)
    deltas[:nsel, 0] = dy / f32(0.1)
    deltas[:nsel, 1] = dh / f32(0.2)
    deltas[:nsel, 2] = dx / f32(0.1)
    deltas[:, 3] = 1.0
    cls_out[:nsel, 0] = gcls_b[sg, 0]
    cls_out[:, 1] = 1.0
    indices[:nsel, 0] = vidx[sm]
    indices[:nsel, 1] = 1
    nn = len(njs)
    indices[K_POS : K_POS + nn, 0] = vidx[njs]
    indices[K_POS : K_POS + nn, 1] = -1
    gt_num = f32((gt[:, 4] != 0.0).sum())
    return (
        deltas, cls_out, indices, gt_num,
        f32(min(pos_cnt, K_POS)), f32(nn),
        np.array([min_iou], f32), np.array([mean_iou], f32),
    )


def kernel(gt_boxes, gt_cls, anchors, valid_anchors_indices):
    gt_boxes = np.asarray(gt_boxes, np.float32)
    gt_cls = np.asarray(gt_cls, np.float32)
    anchors = np.asarray(anchors, np.float32)
    vidx = np.asarray(valid_anchors_indices, np.int32)
    UP, NEG = _prng_consts()
    nc = _get_nc()
    in_maps = [
        {"gtb": np.ascontiguousarray(gt_boxes[b]),
         "anc": np.ascontiguousarray(anchors[b])}
        for b in range(B)
    ]
    res = bass_utils.run_bass_kernel_spmd(nc, in_maps, list(range(B)))
    outs = []
    for b in range(B):
        r = res.results[b]
        outs.append(
            _finish_one(
                gt_boxes[b], gt_cls[b], anchors[b], vidx[b], UP[b], NEG[b],
                np.asarray(r["amax_d"]), np.asarray(r["aidx_d"]),
                np.asarray(r["keyA_d"]), np.asarray(r["gmax_d"]),
            )
        )
    return tuple(np.stack([o[i] for o in outs]) for i in range(8))


# revision 11
# speedup vs baseline: 1.2113x; 1.2113x over previous
"""CTPN target assignment kernel for 8 Trainium2 NeuronCores.

Data-parallel: one image per core. The device computes the dominant
O(G*M) work per image: the full [50, 61440] IoU matrix (tiled as
[128 partitions x 480] over anchors, GT dim unrolled), per-GT row
maxes (gmax), per-anchor column maxes + argmax (amax/aidx), and the
per-anchor GT-argmax key (keyA) that marks "best anchor for GT g"
columns. Those four small per-anchor/per-GT arrays are returned and
the O(pos_cnt) sampling/assembly (top-64 by precomputed threefry
priorities, negative sampling, box regression on <=64 rows) finishes
on host, mirroring the reference bit-exactly on the selection path.
"""

import sys
import numpy as np

if "/opt/trn_rl_repo" not in sys.path:
    sys.path.insert(0, "/opt/trn_rl_repo")

import concourse.bass as bass
import concourse.bacc as bacc
import concourse.mybir as mybir
from concourse.tile import TileContext
from concourse import bass_utils
from concourse.masks import make_identity

B, G, M = 8, 50, 61440
P, F = 128, 480  # anchor tiling: m = p*F + f
K_POS = K_NEG = 64
NEG_T = 256
F07 = float(np.float32(0.7))
ALU = mybir.AluOpType
ACTF = mybir.ActivationFunctionType
DT = mybir.dt


# ---------------- threefry (bit-exact numpy port of jax partitionable PRNG) ----
def _threefry2x32(k0, k1, x0, x1):
    rot = [(13, 15, 26, 6), (17, 29, 16, 24)]
    ks0, ks1 = np.uint32(k0), np.uint32(k1)
    ks = [ks0, ks1, np.uint32(ks0 ^ ks1 ^ np.uint32(0x1BD11BDA))]
    x0 = (x0 + ks0).astype(np.uint32)
    x1 = (x1 + ks1).astype(np.uint32)
    for i in range(5):
        for r in rot[i % 2]:
            x0 = (x0 + x1).astype(np.uint32)
            x1 = ((x1 << np.uint32(r)) | (x1 >> np.uint32(32 - r))).astype(np.uint32)
            x1 = (x1 ^ x0).astype(np.uint32)
        x0 = (x0 + ks[(i + 1) % 3]).astype(np.uint32)
        x1 = (x1 + ks[(i + 2) % 3] + np.uint32(i + 1)).astype(np.uint32)
    return x0, x1


def _tf_bits(k0, k1, n):
    idx = np.arange(n, dtype=np.uint64)
    hi = (idx >> np.uint64(32)).astype(np.uint32)
    lo = (idx & np.uint64(0xFFFFFFFF)).astype(np.uint32)
    b1, b2 = _threefry2x32(k0, k1, hi, lo)
    return (b1 ^ b2).astype(np.uint32)


def _tf_split(k0, k1, num):
    idx = np.arange(num, dtype=np.uint64)
    hi = (idx >> np.uint64(32)).astype(np.uint32)
    lo = (idx & np.uint64(0xFFFFFFFF)).astype(np.uint32)
    b1, b2 = _threefry2x32(k0, k1, hi, lo)
    return np.stack([b1, b2], axis=1)


def _tf_uniform(k0, k1, n):
    bits = _tf_bits(k0, k1, n)
    f = ((bits >> np.uint32(9)) | np.uint32(0x3F800000)).view(np.float32)
    return np.maximum(np.float32(0.0), (f - np.float32(1.0)).astype(np.float32))


_PRNG = None


def _prng_consts():
    """UP[b] = uniform(kp,(G*M,)), neg order = argsort(-UN[b])[:NEG_T]."""
    global _PRNG
    if _PRNG is None:
        keys = _tf_split(np.uint32(0), np.uint32(42), B)
        UP, NEG = [], []
        for b in range(B):
            kp_kn = _tf_split(keys[b][0], keys[b][1], 2)
            UP.append(_tf_uniform(kp_kn[0][0], kp_kn[0][1], G * M))
            un = _tf_uniform(kp_kn[1][0], kp_kn[1][1], M)
            NEG.append(np.argsort(-un, kind="stable")[:NEG_T].astype(np.int64))
        _PRNG = (UP, NEG)
    return _PRNG


# ---------------- device program -------------------------------------------
def _build_program():
    nc = bacc.Bacc("TRN2", target_bir_lowering=False, debug=False)
    gtb = nc.dram_tensor("gtb", [G, 5], DT.float32, kind="ExternalInput")
    anc = nc.dram_tensor("anc", [M, 4], DT.float32, kind="ExternalInput")
    amax_d = nc.dram_tensor("amax_d", [M], DT.float32, kind="ExternalOutput")
    aidx_d = nc.dram_tensor("aidx_d", [M], DT.float32, kind="ExternalOutput")
    keyA_d = nc.dram_tensor("keyA_d", [M], DT.float32, kind="ExternalOutput")
    gmax_d = nc.dram_tensor("gmax_d", [G], DT.float32, kind="ExternalOutput")

    with TileContext(nc) as tc:
        with (
            tc.tile_pool(name="per", bufs=1) as per,
            tc.tile_pool(name="wrk", bufs=3) as wrk,
            tc.tile_pool(name="ps", bufs=2, space="PSUM") as ps,
        ):
            f32 = DT.float32
            packed = per.tile([P, F * 4], f32)
            nc.sync.dma_start(
                packed[:], anc[:].rearrange("(p f) c -> p (f c)", p=P)
            )
            pk3 = packed[:].rearrange("p (f c) -> p f c", c=4)
            AY1 = per.tile([P, F], f32, tag="ay1")
            AX1 = per.tile([P, F], f32, tag="ax1")
            AY2 = per.tile([P, F], f32, tag="ay2")
            AX2 = per.tile([P, F], f32, tag="ax2")
            for t, c in ((AY1, 0), (AX1, 1), (AY2, 2), (AX2, 3)):
                nc.vector.tensor_copy(t[:], pk3[:, :, c])
            Aa = per.tile([P, F], f32, tag="aa")
            tw = wrk.tile([P, F], f32, tag="tw")
            nc.vector.tensor_sub(tw[:], AX2[:], AX1[:])
            nc.vector.tensor_sub(Aa[:], AY2[:], AY1[:])
            nc.vector.tensor_mul(Aa[:], tw[:], Aa[:])

            # gt prep -> [50,8] -> transpose -> broadcast tiles [128,50]
            g50 = per.tile([G, 5], f32, tag="g50")
            nc.sync.dma_start(g50[:], gtb[:])
            ext = per.tile([G, 8], f32, tag="ext")
            nc.vector.memset(ext[:], 0.0)
            for c in range(4):
                nc.vector.tensor_copy(ext[:, c : c + 1], g50[:, c : c + 1])
            w1 = wrk.tile([G, 1], f32, tag="w1")
            nc.vector.tensor_sub(w1[:], g50[:, 3:4], g50[:, 1:2])
            nc.vector.tensor_sub(ext[:, 4:5], g50[:, 2:3], g50[:, 0:1])
            nc.vector.tensor_mul(ext[:, 4:5], w1[:], ext[:, 4:5])
            nc.vector.tensor_scalar(
                ext[:, 5:6], g50[:, 4:5], 0.0, None, ALU.not_equal
            )
            ident = per.tile([P, P], f32, tag="ident")
            make_identity(nc, ident[:])
            extT_ps = ps.tile([8, G], f32, space="PSUM")
            nc.tensor.transpose(extT_ps[:], ext[:], ident[:G, :G])
            t8 = per.tile([8, G], f32, tag="t8")
            nc.vector.tensor_copy(t8[:], extT_ps[:])
            names = ["gy1b", "gx1b", "gy2b", "gx2b", "agb", "validb"]
            bcast = {}
            for k, nm in enumerate(names):
                r0 = per.tile([1, G], f32, tag=nm + "_r0")
                nc.sync.dma_start(r0[:], t8[k : k + 1, :])
                bt = per.tile([P, G], f32, tag=nm)
                nc.gpsimd.partition_broadcast(bt[:], r0[:])
                bcast[nm] = bt
            gy1b, gx1b = bcast["gy1b"], bcast["gx1b"]
            gy2b, gx2b = bcast["gy2b"], bcast["gx2b"]
            agb, validb = bcast["agb"], bcast["validb"]

            iou_big = per.tile([P, F * G], f32, tag="ioubig")
            amax = per.tile([P, F], f32, tag="amax")
            aidx = per.tile([P, F], f32, tag="aidx")
            keyA = per.tile([P, F], f32, tag="keya")
            gcols = per.tile([P, G], f32, tag="gcols")
            nc.vector.memset(amax[:], 0.0)
            nc.vector.memset(aidx[:], 0.0)
            nc.vector.memset(keyA[:], 0.0)

            import os as _os
            _GRUN = int(_os.environ.get('KPROBE_G', G))
            _PH = int(_os.environ.get('KPROBE_PH', 2))
            for g in range(_GRUN):
                sl = slice(g * F, (g + 1) * F)
                iou = iou_big[:, sl]
                mnx = wrk.tile([P, F], f32, tag="mnx")
                nxw = wrk.tile([P, F], f32, tag="nxw")
                mny = wrk.tile([P, F], f32, tag="mny")
                nyh = wrk.tile([P, F], f32, tag="nyh")
                iwp = wrk.tile([P, F], f32, tag="iwp")
                ihp = wrk.tile([P, F], f32, tag="ihp")
                uni = wrk.tile([P, F], f32, tag="uni")
                rcp = wrk.tile([P, F], f32, tag="rcp")
                cmp = wrk.tile([P, F], f32, tag="cmp")
                nc.vector.tensor_scalar(mnx[:], AX2[:], gx2b[:, g : g + 1], None, ALU.min)
                nc.vector.scalar_tensor_tensor(
                    nxw[:], AX1[:], gx1b[:, g : g + 1], mnx[:], ALU.max, ALU.subtract
                )
                nc.vector.tensor_scalar(mny[:], AY2[:], gy2b[:, g : g + 1], None, ALU.min)
                nc.vector.scalar_tensor_tensor(
                    nyh[:], AY1[:], gy1b[:, g : g + 1], mny[:], ALU.max, ALU.subtract
                )
                # iw = relu(-nxw), ih = relu(-nyh) on ACT (parallel engine)
                nc.scalar.activation(iwp[:], nxw[:], ACTF.Relu, 0.0, -1.0)
                nc.scalar.activation(ihp[:], nyh[:], ACTF.Relu, 0.0, -1.0)
                inter = mnx  # reuse
                nc.gpsimd.tensor_mul(inter[:], iwp[:], ihp[:])
                nc.vector.scalar_tensor_tensor(
                    uni[:], Aa[:], agb[:, g : g + 1], inter[:], ALU.add, ALU.subtract
                )
                nc.vector.reciprocal_approx_fast(rcp[:], uni[:])
                nc.vector.scalar_tensor_tensor(
                    iou[:], inter[:], validb[:, g : g + 1], rcp[:], ALU.mult, ALU.mult
                )
                nc.vector.tensor_reduce(
                    gcols[:, g : g + 1], iou[:], mybir.AxisListType.X, ALU.max
                )
                nc.vector.tensor_tensor(cmp[:], iou[:], amax[:], ALU.is_gt)
                nc.vector.tensor_max(amax[:], amax[:], iou[:])
                nc.vector.scalar_tensor_tensor(
                    aidx[:], cmp[:], float(g), aidx[:], ALU.mult, ALU.max
                )

            if _PH < 2:
                nc.sync.dma_start(amax_d[:].rearrange("(p f) -> p f", p=P), amax[:])
                nc.sync.dma_start(aidx_d[:].rearrange("(p f) -> p f", p=P), aidx[:])
                nc.sync.dma_start(keyA_d[:].rearrange("(p f) -> p f", p=P), keyA[:])
                nc.sync.dma_start(gmax_d[:].rearrange("(p f) -> p f", f=1), gcols[:G, 0:1])
                nc.compile()
                return nc
            # gmax: [128,50] -> T -> [50,128] -> reduce X
            gT_ps = ps.tile([G, P], f32, space="PSUM")
            nc.tensor.transpose(gT_ps[:], gcols[:], ident[:])
            gmax = per.tile([G, 1], f32, tag="gmax")
            nc.vector.tensor_reduce(gmax[:], gT_ps[:], mybir.AxisListType.X, ALU.max)
            # broadcast gmax -> [128,50]; select -1 for invalid rows
            gmT_ps = ps.tile([1, G], f32, space="PSUM")
            nc.tensor.transpose(gmT_ps[:], gmax[:], ident[:G, :G])
            grow = per.tile([1, G], f32, tag="grow")
            nc.vector.tensor_copy(grow[:], gmT_ps[:])
            gmaxb = per.tile([P, G], f32, tag="gmaxb")
            nc.gpsimd.partition_broadcast(gmaxb[:], grow[:])
            gsel = per.tile([P, G], f32, tag="gsel")
            nc.vector.tensor_mul(gsel[:], gmaxb[:], validb[:])
            nc.vector.scalar_tensor_tensor(
                gsel[:], validb[:], -1.0, gsel[:], ALU.add, ALU.add
            )
            for g in range(_GRUN):
                e = wrk.tile([P, F], f32, tag="e2")
                nc.vector.tensor_scalar(
                    e[:], iou_big[:, g * F : (g + 1) * F], gsel[:, g : g + 1], None,
                    ALU.is_equal,
                )
                nc.vector.scalar_tensor_tensor(
                    keyA[:], e[:], float(g + 1), keyA[:], ALU.mult, ALU.max
                )

            nc.sync.dma_start(amax_d[:].rearrange("(p f) -> p f", p=P), amax[:])
            nc.sync.dma_start(aidx_d[:].rearrange("(p f) -> p f", p=P), aidx[:])
            nc.sync.dma_start(keyA_d[:].rearrange("(p f) -> p f", p=P), keyA[:])
            nc.sync.dma_start(gmax_d[:].rearrange("(p f) -> p f", f=1), gmax[:])
    nc.compile()
    return nc


_NC = None


def _get_nc():
    global _NC
    if _NC is None:
        _NC = _build_program()
    return _NC


# ---------------- host assembly (mirrors verified golden model) -------------
def _finish_one(gt, gcls_b, anc, vidx, up, negids, amax, aidx, keyA, gmax):
    f32 = np.float32
    gt = np.asarray(gt, f32)
    hot = amax >= f32(F07)
    entA = keyA > 0
    gA = np.where(entA, keyA - 1, 0).astype(np.int64)
    vA = gmax[gA]
    dup = entA & hot & (vA == amax)
    mB = np.where(hot)[0]
    mA = np.where(entA & ~dup)[0]
    ents_g = np.concatenate([aidx[mB].astype(np.int64), gA[mA]])
    ents_m = np.concatenate([mB, mA])
    ents_v = np.concatenate([amax[mB], vA[mA]])
    pos_cnt = len(ents_g)
    min_iou = ents_v.min()
    mean_iou = f32(ents_v.sum(dtype=f32) / f32(pos_cnt))
    prio = up[ents_g * M + ents_m]
    order = np.argsort(-prio, kind="stable")[:K_POS]
    sg, sm = ents_g[order], ents_m[order]
    nsel = len(sg)  # == 64 (pos_cnt >= 64 for this data)

    flags = (amax[negids] < f32(0.5)) & ~(entA[negids] | hot[negids])
    njs = negids[flags][:K_NEG]

    deltas = np.zeros((128, 4), f32)
    cls_out = np.zeros((128, 2), f32)
    indices = np.zeros((128, 2), np.int32)
    a = anc[sm]
    gg = gt[sg]
    hh = (a[:, 2] - a[:, 0]).astype(f32)
    gth = (gg[:, 2] - gg[:, 0]).astype(f32)
    dy = ((((gg[:, 2] + gg[:, 0]) * f32(0.5)) - ((a[:, 2] + a[:, 0]) * f32(0.5))) / hh).astype(f32)
    dh = np.log((gth / hh).astype(f32)).astype(f32)
    ww = (a[:, 3] - a[:, 1]).astype(f32)
    dx = (((((gg[:, 3] + gg[:, 1]) * f32(0.5)) - ((a[:, 3] + a[:, 1]) * f32(0.5))) * f32(2.0)) / ww).astype(f32)
    deltas[:nsel, 0] = dy / f32(0.1)
    deltas[:nsel, 1] = dh / f32(0.2)
    deltas[:nsel, 2] = dx / f32(0.1)
    deltas[:, 3] = 1.0
    cls_out[:nsel, 0] = gcls_b[sg, 0]
    cls_out[:, 1] = 1.0
    indices[:nsel, 0] = vidx[sm]
    indices[:nsel, 1] = 1
    nn = len(njs)
    indices[K_POS : K_POS + nn, 0] = vidx[njs]
    indices[K_POS : K_POS + nn, 1] = -1
    gt_num = f32((gt[:, 4] != 0.0).sum())
    return (
        deltas, cls_out, indices, gt_num,
        f32(min(pos_cnt, K_POS)), f32(nn),
        np.array([min_iou], f32), np.array([mean_iou], f32),
    )


def kernel(gt_boxes, gt_cls, anchors, valid_anchors_indices):
    gt_boxes = np.asarray(gt_boxes, np.float32)
    gt_cls = np.asarray(gt_cls, np.float32)
    anchors = np.asarray(anchors, np.float32)
    vidx = np.asarray(valid_anchors_indices, np.int32)
    UP, NEG = _prng_consts()
    nc = _get_nc()
    in_maps = [
        {"gtb": np.ascontiguousarray(gt_boxes[b]),
         "anc": np.ascontiguousarray(anchors[b])}
        for b in range(B)
    ]
    res = bass_utils.run_bass_kernel_spmd(nc, in_maps, list(range(B)))
    outs = []
    for b in range(B):
        r = res.results[b]
        outs.append(
            _finish_one(
                gt_boxes[b], gt_cls[b], anchors[b], vidx[b], UP[b], NEG[b],
                np.asarray(r["amax_d"]), np.asarray(r["aidx_d"]),
                np.asarray(r["keyA_d"]), np.asarray(r["gmax_d"]),
            )
        )
    return tuple(np.stack([o[i] for o in outs]) for i in range(8))
